# revision 1
# baseline (speedup 1.0000x reference)
# Multi-head attention (N=4, S=2048, E=512, H=8, D=64) on 8 NeuronCores.
#
# Sharding: core c -> (batch n = c//2, query half qh = c%2). Each core
# computes attention for its 1024 query rows against the full 2048 keys of
# its batch, all 8 heads, and the full output projection for its rows, so
# outputs are disjoint and no collectives are needed.
#
# Host-side weight folding (pure weight algebra, done once):
#   A  = Wq^T @ Wk / 8          scores = (Xq @ A) @ Xk^T  (raw K, one proj)
#   M_h = Wv^T @ Wo[:, h]^T     out += (attn @ Xv_h) @ M_h (Wv applied post)
#   btot = bo + Wo @ tile(bv,8) exact because attention rows sum to 1
#   bk-term cancels in softmax (constant over k); bq-term handled via a
#   per-k bias correction (zero for this problem's inputs).
#
# Device per core:
#   - PE-transpose Q,K chunks (bf16) to [e, s] layout
#   - XqA^T = A^T-proj of Q^T per head
#   - scores^T[k,q] = K_h^T.T @ XqA_h^T  (PSUM f32)
#   - exp on ACT straight from PSUM with per-partition mask bias -> bf16
#   - AV^T = V_aug.T @ exp^T with a ones column in V giving softmax
#     denominators as row 64; normalize with DVE + DMA-replicated recip
#   - out[q,:] = sum_h AVT_h.T @ M_h (+btot), accumulated in PSUM

import numpy as np
import ml_dtypes

import concourse.bass as bass
import concourse.tile as tile
from concourse import bacc, mybir
from concourse.bass_utils import run_bass_kernel_spmd
from concourse.masks import make_identity

F32 = mybir.dt.float32
BF16 = mybir.dt.bfloat16
I32 = mybir.dt.int32

H = 8
D = 64
E = 512
N_CORES = 8
FULL_N, FULL_S = 4, 2048
SQ, SK = 1024, 2048  # per-core query rows / key rows
MASK_BIAS = -1.25e8  # == -1e9 / sqrt(64), applied pre-softmax


USE_FAST_RECIP = False  # custom-DVE ops produce garbage under this runtime


def _emit(tc, t, SQ, SK, has_qbias, stop_phase=99):
    nc = tc.nc
    NQC = SQ // 128           # query chunks (transpose granularity)
    NKC = SK // 128           # key chunks
    QGS = min(512, SQ)        # q group size for matmul free dim
    NQG = SQ // QGS
    HALF = max(1, NKC // 2)   # k-tiles per exp half-tile
    F32R = mybir.dt.float32r
    sub, mult, add = (mybir.AluOpType.subtract, mybir.AluOpType.mult,
                      mybir.AluOpType.add)

    with (
        tc.tile_pool(name="singles", bufs=1) as singles,
        tc.tile_pool(name="stage", bufs=3) as stage,
        tc.tile_pool(name="expp", bufs=6) as expp,
        tc.tile_pool(name="small", bufs=2) as small,
        tc.tile_pool(name="outp", bufs=2) as outp,
        tc.tile_pool(name="p_sc", bufs=3, space="PSUM") as p_sc,
        tc.tile_pool(name="p_misc", bufs=2, space="PSUM") as p_misc,
    ):
        # ---- constants / weights ----
        ident = singles.tile([128, 128], BF16)
        make_identity(nc, ident)

        a_sb = singles.tile([128, D], BF16)
        nc.sync.dma_start(a_sb, t["a2"][:])
        m_sb = singles.tile([128, 4, E], BF16)   # loaded later, needed by post
        btot_rep = singles.tile([128, E], F32)   # broadcast-loaded later

        # mask -> additive bias, [128, NKC] with k = kt*128 + p
        mask_i = singles.tile([128, NKC], I32)
        nc.scalar.dma_start(mask_i, t["mask"][:].rearrange("(kt p) -> p kt", p=128))
        mask_f = singles.tile([128, NKC], F32)
        nc.vector.tensor_copy(mask_f, mask_i)
        mbias = singles.tile([128, NKC], F32)
        # (mask - 1) * (-MASK_BIAS):  mask=0 -> MASK_BIAS, mask=1 -> 0
        nc.vector.tensor_scalar(out=mbias, in0=mask_f, scalar1=1.0,
                                scalar2=-MASK_BIAS, op0=sub, op1=mult)

        # ---- persistent tensors ----
        ones1f = singles.tile([1, D], F32)
        nc.vector.memset(ones1f, 1.0)
        ones1 = singles.tile([1, D], F32R)       # f32r needs a rounding producer
        nc.vector.tensor_copy(ones1, ones1f)
        qt = singles.tile([128, 4, SQ], BF16)    # query^T: e=fc*128+p
        kt = singles.tile([128, 4, SK], BF16)    # key^T
        vt = singles.tile([128, NKC, H, D + 1], BF16)  # value + ones col
        xqa = singles.tile([128, 4, SQ], BF16)   # (Xq @ A)^T per head
        # normalized (attn @ V)^T: head pair layout, odd heads at base 64
        avt = singles.tile([128, 4, SQ], BF16)

        # ---- chunk loader: load f32, cast bf16, PE-transpose into dst ----
        def load_transposed_chunk(src, dst, c, queue):
            raw = stage.tile([128, E], F32, tag="ld")
            queue.dma_start(raw, src[c * 128:(c + 1) * 128, :])
            cast = stage.tile([128, E], BF16, tag="cast")
            nc.vector.tensor_copy(cast, raw)
            tp = p_sc.tile([128, 4, 128], BF16, tag="sc")
            for eg in range(4):
                nc.tensor.transpose(tp[:, eg, :],
                                    cast[:, eg * 128:(eg + 1) * 128], ident)
            nc.vector.tensor_copy(dst[:, :, c * 128:(c + 1) * 128], tp)

        # ---- Q: load + transpose, then the A-projection per head ----
        for c in range(NQC):
            load_transposed_chunk(t["query"][:], qt, c, nc.sync)
        for h in range(H):
            bp, fc = 64 * (h % 2), h // 2
            for g in range(NQG):
                ps = p_sc.tile([128, QGS], F32, tag="sc")
                nc.tensor.matmul(ps[bp:bp + 64, :], lhsT=a_sb[bp:bp + 64, :],
                                 rhs=qt[bp:bp + 64, fc, g * QGS:(g + 1) * QGS],
                                 start=True, stop=True)
                nc.vector.tensor_copy(xqa[bp:bp + 64, fc, g * QGS:(g + 1) * QGS],
                                      ps[bp:bp + 64, :])

        # ---- optional exact bq correction: per-(h,k) additive bias ----
        # scores^T gains (Xk_h @ (Wk^T bq / 8))[k], constant over q.
        if has_qbias:
            for c in range(NKC):
                load_transposed_chunk(t["key"][:], kt, c, nc.gpsimd)
            w2 = singles.tile([128, 1], BF16)
            nc.sync.dma_start(w2, t["w2"][:])
            hbias = []
            for h in range(H):
                bp, fc = 64 * (h % 2), h // 2
                row = small.tile([1, SK], F32, tag="hb_row")
                for g in range(SK // 512):
                    ps = p_misc.tile([128, 512], F32, tag="ps")
                    nc.tensor.matmul(ps[0:1, :], lhsT=w2[bp:bp + 64, :],
                                     rhs=kt[bp:bp + 64, fc, g * 512:(g + 1) * 512],
                                     start=True, stop=True)
                    nc.vector.tensor_copy(row[:, g * 512:(g + 1) * 512],
                                          ps[0:1, :])
                hb = singles.tile([128, NKC], F32, name=f"hbias{h}")
                nc.gpsimd.dma_start(hb, row[0, :].rearrange("(kt p) -> p kt",
                                                            p=128))
                nc.vector.tensor_tensor(out=hb, in0=hb, in1=mbias, op=add)
                hbias.append(hb)
        else:
            hbias = [mbias] * H

        # ---- per head-pair: scores^T -> exp -> AV^T, row-group alternated ---
        # Even/odd heads live at SBUF partition bases 0/64, so interleaving a
        # head pair's score matmuls alternates PE row groups: LDWEIGHTS pulls
        # ahead and the two matmuls execute concurrently on the array halves.
        # K-chunk transposes are folded into pair 0; attn@V matmuls of the
        # previous pair's heads are folded into later pairs for PE smoothness.
        av_state = {}

        def av_alloc(h, pool=None, tag="ps"):
            pool = pool or p_misc
            av_state[h] = [pool.tile([128, QGS], F32, tag=tag,
                                     name=f"av{h}g{g}") for g in range(NQG)]

        def av_mm(h, exA, exB, c):
            ex = exA if c < HALF else exB
            for g in range(NQG):
                nc.tensor.matmul(av_state[h][g][0:D + 1, :],
                                 lhsT=vt[:, c, h, :],
                                 rhs=ex[:, c % HALF, g * QGS:(g + 1) * QGS],
                                 start=(c == 0), stop=(c == NKC - 1))

        def av_norm(h):
            # copy AV+denominator out of PSUM (f32r rounding copy frees the
            # accumulator early), reciprocal the denominator row, broadcast it
            # across partitions with a rank-1 f32r matmul, multiply, store.
            # Odd heads hop to partition base 64 of avt via a tiny SBUF DMA so
            # the output projection can alternate PE row groups.
            fc = h // 2
            for g in range(NQG):
                ps = av_state[h][g]
                avsb = small.tile([128, QGS], F32R, tag="avsb")
                nc.vector.tensor_copy(avsb[0:D + 1, :], ps[0:D + 1, :])
                rec = small.tile([1, QGS], F32, tag="rec")
                nc.vector.reciprocal(rec, avsb[D:D + 1, :].bitcast(F32))
                recr = small.tile([1, QGS], F32R, tag="recr")
                nc.vector.tensor_copy(recr, rec)
                pb = p_sc.tile([128, QGS], F32, tag="sc")
                nc.tensor.matmul(pb[0:D, :], lhsT=ones1, rhs=recr,
                                 start=True, stop=True)
                gsl = slice(g * QGS, (g + 1) * QGS)
                if h % 2 == 0:
                    nc.vector.tensor_tensor(out=avt[0:D, fc, gsl],
                                            in0=avsb[0:D, :].bitcast(F32),
                                            in1=pb[0:D, :], op=mult)
                else:
                    avtmp = small.tile([64, QGS], BF16, tag="avtmp")
                    nc.vector.tensor_tensor(out=avtmp,
                                            in0=avsb[0:D, :].bitcast(F32),
                                            in1=pb[0:D, :], op=mult)
                    nc.gpsimd.dma_start(avt[64:64 + D, fc, gsl], avtmp)
            del av_state[h]

        def emit_half(p, half, av_head=None, av_tiles=None, ktrans=False,
                      norm_head=None):
            h0, h1 = 2 * p, 2 * p + 1
            exs = [expp.tile([128, HALF, SQ], BF16, tag="exp",
                             name=f"exp_{h}_{half}") for h in (h0, h1)]
            if av_head is not None and av_head not in av_state:
                av_alloc(av_head)
            for kt_l in range(HALF):
                kt_i = half * HALF + kt_l
                if ktrans:
                    load_transposed_chunk(t["key"][:], kt, kt_i, nc.gpsimd)
                pss = [p_sc.tile([128, SQ], F32, tag="sc", name=f"sc{j}")
                       for j in range(2)]
                for g in range(NQG):
                    for j, h in enumerate((h0, h1)):
                        bp, fc = 64 * (h % 2), h // 2
                        nc.tensor.matmul(
                            pss[j][:, g * QGS:(g + 1) * QGS],
                            lhsT=kt[bp:bp + 64, fc,
                                    kt_i * 128:(kt_i + 1) * 128],
                            rhs=xqa[bp:bp + 64, fc, g * QGS:(g + 1) * QGS],
                            start=True, stop=True)
                if av_head is not None:
                    for c in (2 * kt_l, 2 * kt_l + 1):
                        av_mm(av_head, av_tiles[0], av_tiles[1], c)
                for j, h in enumerate((h0, h1)):
                    nc.scalar.activation(out=exs[j][:, kt_l, :], in_=pss[j],
                                         func=mybir.ActivationFunctionType.Exp,
                                         bias=hbias[h][:, kt_i:kt_i + 1],
                                         scale=1.0)
                if kt_l == 0 and norm_head is not None:
                    # deferred: normalizing the head finished in the previous
                    # half here keeps its DVE chain off the PE critical path
                    av_norm(norm_head)
            return exs

        halves = {}

        def set_halves(p, half, exs):
            halves[(2 * p, half)], halves[(2 * p + 1, half)] = exs

        def pop_head(h):
            return halves.pop((h, 0)), halves.pop((h, 1))

        set_halves(0, 0, emit_half(0, 0, ktrans=not has_qbias))
        set_halves(0, 1, emit_half(0, 1, ktrans=not has_qbias))
        # V: load + cast while pair-1 scores run (needed first by AV(h0))
        nc.gpsimd.dma_start(btot_rep,
                            t["btot"][:][None, :].to_broadcast([128, E]))
        nc.vector.memset(vt[:, :, :, D:D + 1], 1.0)
        for c in range(NKC):
            raw = stage.tile([128, E], F32, tag="ld")
            nc.sync.dma_start(raw, t["value"][c * 128:(c + 1) * 128, :])
            nc.vector.tensor_copy(vt[:, c, :, 0:D],
                                  raw.rearrange("p (h d) -> p h d", h=H))
        pending = None
        for p in range(1, H // 2):
            hh = 2 * (p - 1)
            ta, tb = pop_head(hh)
            set_halves(p, 0, emit_half(p, 0, av_head=hh, av_tiles=(ta, tb),
                                       norm_head=pending))
            ta, tb = pop_head(hh + 1)
            set_halves(p, 1, emit_half(p, 1, av_head=hh + 1,
                                       av_tiles=(ta, tb), norm_head=hh))
            pending = hh + 1
        nc.sync.dma_start(m_sb, t["m2"][:])
        t6, t7 = pop_head(H - 2), pop_head(H - 1)
        av_norm(pending)

        # ---- output projection pass 1: head slots 0..2 are done; fold them
        # into a bf16 accumulator while the last exps drain on ACT.
        out_acc = singles.tile([128, NQC, E], BF16)
        for q_i in range(SQ // 128):
            qs = slice(q_i * 128, (q_i + 1) * 128)
            psA = p_misc.tile([128, E], F32, tag="ps")
            psB = p_sc.tile([128, max(SQ, E)], F32, tag="sc")
            for j in range(3):
                nc.tensor.matmul(psA, lhsT=avt[0:D, j, qs],
                                 rhs=m_sb[0:D, j, :],
                                 start=(j == 0), stop=(j == 2))
                nc.tensor.matmul(psB[:, 0:E], lhsT=avt[64:64 + D, j, qs],
                                 rhs=m_sb[64:64 + D, j, :],
                                 start=(j == 0), stop=(j == 2))
            # DVE has one PSUM read port: copy psA out, then add psB
            nc.vector.tensor_copy(out_acc[:, q_i, :], psA)
            nc.vector.tensor_tensor(out=out_acc[:, q_i, :], in0=psB[:, 0:E],
                                    in1=out_acc[:, q_i, :], op=add)
        av_alloc(H - 2)
        av_alloc(H - 1, pool=p_sc, tag="sc")
        for c in range(NKC):
            av_mm(H - 2, t6[0], t6[1], c)
            av_mm(H - 1, t7[0], t7[1], c)
        av_norm(H - 2)
        av_norm(H - 1)

        # ---- output projection pass 2: last head pair + bias, then store ---
        for q_i in range(SQ // 128):
            qs = slice(q_i * 128, (q_i + 1) * 128)
            psA = p_misc.tile([128, E], F32, tag="ps")
            psB = p_sc.tile([128, max(SQ, E)], F32, tag="sc")
            nc.tensor.matmul(psA, lhsT=avt[0:D, 3, qs], rhs=m_sb[0:D, 3, :],
                             start=True, stop=True)
            nc.tensor.matmul(psB[:, 0:E], lhsT=avt[64:64 + D, 3, qs],
                             rhs=m_sb[64:64 + D, 3, :], start=True, stop=True)
            ob = outp.tile([128, E], F32, tag="ob")
            nc.vector.tensor_tensor(out=ob, in0=psA, in1=btot_rep, op=add)
            nc.vector.tensor_tensor(out=ob, in0=psB[:, 0:E], in1=ob, op=add)
            ob2 = outp.tile([128, E], F32, tag="ob2")
            nc.vector.tensor_tensor(out=ob2, in0=ob, in1=out_acc[:, q_i, :],
                                    op=add)
            nc.sync.dma_start(t["out"][qs, :], ob2)


def build_module(SQ=SQ, SK=SK, has_qbias=False, stop_phase=99):
    nc = bacc.Bacc()
    t = {
        "query": nc.dram_tensor("query", [SQ, E], F32, kind="ExternalInput"),
        "key": nc.dram_tensor("key", [SK, E], F32, kind="ExternalInput"),
        "value": nc.dram_tensor("value", [SK, E], F32, kind="ExternalInput"),
        "mask": nc.dram_tensor("mask", [SK], I32, kind="ExternalInput"),
        "a2": nc.dram_tensor("a2", [128, D], BF16, kind="ExternalInput"),
        "m2": nc.dram_tensor("m2", [128, 4, E], BF16, kind="ExternalInput"),
        "btot": nc.dram_tensor("btot", [E], F32, kind="ExternalInput"),
        "out": nc.dram_tensor("out", [SQ, E], F32, kind="ExternalOutput"),
    }
    if has_qbias:
        t["w2"] = nc.dram_tensor("w2", [128, 1], BF16, kind="ExternalInput")
    with tile.TileContext(nc) as tc:
        _emit(tc, t, SQ, SK, has_qbias, stop_phase)
    nc.compile()
    return nc


_MODULE_CACHE = {}


def _get_module(SQ, SK, has_qbias):
    key = (SQ, SK, has_qbias)
    if key not in _MODULE_CACHE:
        _MODULE_CACHE[key] = build_module(SQ, SK, has_qbias)
    return _MODULE_CACHE[key]


def _fold_weights(Wq, Wk, Wv, Wo, bv, bo):
    Wq, Wk, Wv, Wo = (np.asarray(w, np.float64) for w in (Wq, Wk, Wv, Wo))
    A = (Wq.T @ Wk) / np.sqrt(np.float64(D))
    a2 = np.concatenate([A, A], axis=0).astype(ml_dtypes.bfloat16)  # [128, 64]
    Ms = [Wv.T @ Wo[:, h * D:(h + 1) * D].T for h in range(H)]
    # head-pair packing: head h at partitions 64*(h%2) .. +64, free slot h//2
    m2 = np.zeros((128, 4, E), np.float64)
    for h in range(H):
        m2[64 * (h % 2):64 * (h % 2) + D, h // 2, :] = Ms[h]
    m2 = m2.astype(ml_dtypes.bfloat16)
    btot = (np.asarray(bo, np.float64)
            + Wo @ np.tile(np.asarray(bv, np.float64), H)).astype(np.float32)
    return a2, m2, btot


def _run(inputs, trace=False):
    query = np.asarray(inputs["query"], np.float32)
    key = np.asarray(inputs["key"], np.float32)
    value = np.asarray(inputs["value"], np.float32)
    mask = np.asarray(inputs["mask"])
    a2, m2, btot = _fold_weights(inputs["Wq"], inputs["Wk"], inputs["Wv"],
                                 inputs["Wo"], inputs["bv"], inputs["bo"])
    bq = np.asarray(inputs["bq"], np.float64)
    bk = np.asarray(inputs["bk"], np.float64)  # noqa: F841  (cancels in softmax)
    has_qbias = bool(np.any(bq != 0))
    w2 = None
    if has_qbias:
        w2v = (np.asarray(inputs["Wk"], np.float64).T @ bq) / np.sqrt(float(D))
        w2 = np.concatenate([w2v, w2v]).reshape(128, 1).astype(ml_dtypes.bfloat16)

    n_batch, S = query.shape[0], query.shape[1]
    sq = S // 2
    nc = _get_module(sq, S, has_qbias)

    in_maps = []
    for c in range(N_CORES):
        n, qh = divmod(c, 2)
        m = {
            "query": np.ascontiguousarray(query[n, qh * sq:(qh + 1) * sq, :]),
            "key": np.ascontiguousarray(key[n]),
            "value": np.ascontiguousarray(value[n]),
            "mask": np.ascontiguousarray(mask[n, 0, 0, :].astype(np.int32)),
            "a2": a2, "m2": m2, "btot": btot,
        }
        if has_qbias:
            m["w2"] = w2
        in_maps.append(m)

    res = run_bass_kernel_spmd(nc, in_maps, core_ids=list(range(N_CORES)),
                               trace=trace)
    out = np.empty((n_batch, S, E), np.float32)
    for c, r in enumerate(res.results):
        n, qh = divmod(c, 2)
        out[n, qh * sq:(qh + 1) * sq, :] = r["out"]
    return out, res


def kernel(**inputs) -> np.ndarray:
    out, _ = _run(inputs, trace=False)
    return out



# revision 16
# speedup vs baseline: 1.0016x; 1.0016x over previous
# Multi-head attention (N=4, S=2048, E=512, H=8, D=64) on 8 NeuronCores.
#
# Sharding: core c -> (batch n = c//2, query half qh = c%2). Each core
# computes attention for its 1024 query rows against the (compacted) keys of
# its batch, all 8 heads, and the full output projection for its rows, so
# outputs are disjoint and no collectives are needed.
#
# Key compaction: the mask is per-(batch, key) broadcast over queries/heads,
# and masked keys get exp(-1.25e8) == 0 exactly in f32, contributing nothing
# to the numerator or denominator. So keys with mask==0 are dropped on the
# host; all batches pad to a common chunk-multiple length SKC with pad keys
# biased to -inf. This halves score/exp/attn@V work for ~50% masks.
#
# Host-side weight folding (pure weight algebra, done once):
#   A  = Wq^T @ Wk / 8          scores = (Xq @ A) @ Xk^T  (raw K, one proj)
#   M_h = Wv^T @ Wo[:, h]^T     out += (attn @ Xv_h) @ M_h (Wv applied post)
#   btot = bo + Wo @ tile(bv,8) exact because attention rows sum to 1
#   bk-term cancels in softmax (constant over k); bq-term handled via a
#   per-k bias correction (zero for this problem's inputs).
#
# Device per core (head pair p owns heads 2p, 2p+1 at partition bases 0/64):
#   - PE-transpose Q,K chunks (bf16) to [e, s] layout
#   - xqa = blockdiag(A,A)^T-proj of Q^T per head pair (128-contraction)
#   - per (head, k-chunk): scores^T[k,q] on PE -> exp on ACT straight from
#     PSUM with per-partition mask bias -> bf16 ex tiles
#   - attn@V accumulates per chunk at 1-chunk lag (V carries a ones column
#     so softmax denominators ride along as PSUM row 64)
#   - normalization: denominator row DMA-rearranged to [128, 8] so the DVE
#     reciprocal runs on all lanes, broadcast back by DMA, one DVE multiply
#   - out[q,:] = sum_fc avt_fc^T @ M_fc (+btot): 128-contraction PSUM
#     accumulation over all four head pairs, one DVE add, DMA out

import numpy as np
import ml_dtypes

import concourse.bass as bass
import concourse.tile as tile
from concourse import bacc, mybir
from concourse.bass_utils import run_bass_kernel_spmd
from concourse.masks import make_identity

F32 = mybir.dt.float32
BF16 = mybir.dt.bfloat16
I32 = mybir.dt.int32

H = 8
D = 64
E = 512
N_CORES = 8
SQ = 1024             # per-core query rows
MASK_BIAS = -1.25e8   # == -1e9 / sqrt(64), applied pre-softmax


def _emit(tc, t, SQ, SK, has_qbias):
    nc = tc.nc
    NQC = SQ // 128           # query chunks (transpose granularity)
    NKC = SK // 128           # key chunks
    QGS = 512                 # q group size for matmul free dim (1 PSUM bank)
    NQG = SQ // QGS
    sub, mult, add = (mybir.AluOpType.subtract, mybir.AluOpType.mult,
                      mybir.AluOpType.add)

    with (
        tc.tile_pool(name="singles", bufs=1) as singles,
        tc.tile_pool(name="stage", bufs=3) as stage,
        tc.tile_pool(name="expp", bufs=4) as expp,
        tc.tile_pool(name="small", bufs=4) as small,
        tc.tile_pool(name="outp", bufs=2) as outp,
        tc.tile_pool(name="p_sc", bufs=2, space="PSUM") as p_sc,
        tc.tile_pool(name="p_av", bufs=2, space="PSUM") as p_av,
    ):
        # ---- constants / weights ----
        ident = singles.tile([128, 128], BF16)
        make_identity(nc, ident)

        a2 = singles.tile([128, 128], BF16)       # blockdiag(A, A)
        nc.sync.dma_start(a2, t["a2"][:])
        m_sb = singles.tile([128, 4, E], BF16)
        nc.sync.dma_start(m_sb, t["m2"][:])
        btot_rep = singles.tile([128, E], F32)
        nc.gpsimd.dma_start(btot_rep,
                            t["btot"][:][None, :].to_broadcast([128, E]))

        # mask -> additive bias, [128, NKC] with k = kt*128 + p
        mask_i = singles.tile([128, NKC], I32)
        nc.scalar.dma_start(mask_i, t["mask"][:].rearrange("(kt p) -> p kt", p=128))
        mask_f = singles.tile([128, NKC], F32)
        nc.vector.tensor_copy(mask_f, mask_i)
        mbias = singles.tile([128, NKC], F32)
        # (mask - 1) * (-MASK_BIAS):  mask=0 -> MASK_BIAS, mask=1 -> 0
        nc.vector.tensor_scalar(out=mbias, in0=mask_f, scalar1=1.0,
                                scalar2=-MASK_BIAS, op0=sub, op1=mult)

        # ---- persistent tensors ----
        F32R = mybir.dt.float32r
        ones1f = singles.tile([1, D], F32)
        nc.vector.memset(ones1f, 1.0)
        ones1 = singles.tile([1, D], F32R)       # f32r needs a rounding producer
        nc.vector.tensor_copy(ones1, ones1f)
        qt = singles.tile([128, 4, SQ], BF16)    # query^T: e=fc*128+p
        kt = singles.tile([128, 4, SK], BF16)    # key^T (compacted)
        vt = singles.tile([128, NKC, H, D + 1], BF16)  # value + ones col
        xqa = singles.tile([128, 4, SQ], BF16)   # (Xq @ A)^T per head
        # normalized (attn @ V)^T: head pair layout, odd heads at base 64
        avt = singles.tile([128, 4, SQ], BF16)

        # ---- chunk loader: load f32, cast bf16, PE-transpose into dst ----
        def load_transposed_chunk(src, dst, c, queue, cast_eng):
            raw = stage.tile([128, E], F32, tag="ld")
            queue.dma_start(raw, src[c * 128:(c + 1) * 128, :])
            cast = stage.tile([128, E], BF16, tag="cast")
            cast_eng.tensor_copy(cast, raw)
            tp = p_av.tile([128, 4, 128], BF16, tag="av")
            for eg in range(4):
                nc.tensor.transpose(tp[:, eg, :],
                                    cast[:, eg * 128:(eg + 1) * 128], ident)
            nc.vector.tensor_copy(dst[:, :, c * 128:(c + 1) * 128], tp)

        # ---- Q: load + transpose, then blockdiag-A projection per pair ----
        for c in range(NQC):
            load_transposed_chunk(t["query"][:], qt, c, nc.sync, nc.vector)
        for fc in range(4):
            for g in range(NQG):
                gsl = slice(g * QGS, (g + 1) * QGS)
                ps = p_av.tile([128, QGS], F32, tag="av")
                nc.tensor.matmul(ps, lhsT=a2, rhs=qt[:, fc, gsl],
                                 start=True, stop=True)
                nc.vector.tensor_copy(xqa[:, fc, gsl], ps)

        # ---- optional exact bq correction: per-(h,k) additive bias ----
        # scores^T gains (Xk_h @ (Wk^T bq / 8))[k], constant over q.
        if has_qbias:
            for c in range(NKC):
                load_transposed_chunk(t["key"][:], kt, c, nc.gpsimd, nc.gpsimd)
            w2 = singles.tile([128, 1], BF16)
            nc.sync.dma_start(w2, t["w2"][:])
            hbias = []
            for h in range(H):
                bp, fc = 64 * (h % 2), h // 2
                row = small.tile([1, SK], F32, tag="hb_row")
                for g in range(SK // 512):
                    ps = p_av.tile([128, 512], F32, tag="av")
                    nc.tensor.matmul(ps[0:1, :], lhsT=w2[bp:bp + 64, :],
                                     rhs=kt[bp:bp + 64, fc, g * 512:(g + 1) * 512],
                                     start=True, stop=True)
                    nc.vector.tensor_copy(row[:, g * 512:(g + 1) * 512],
                                          ps[0:1, :])
                hb = singles.tile([128, NKC], F32, name=f"hbias{h}")
                nc.gpsimd.dma_start(hb, row[0, :].rearrange("(kt p) -> p kt",
                                                            p=128))
                nc.vector.tensor_tensor(out=hb, in0=hb, in1=mbias, op=add)
                hbias.append(hb)
        else:
            hbias = [mbias] * H

        # ---- K: load + transpose (before the pair loop so the transpose
        # PSUM tiles never contend with the AV accumulators; pair-0 scores
        # start as soon as chunk 0 lands in kt) ----
        if not has_qbias:
            for c in range(NKC):
                load_transposed_chunk(t["key"][:], kt, c, nc.gpsimd,
                                      nc.gpsimd)

        # ---- V: load + cast + ones column (consumed from pair 0's AV) ----
        nc.vector.memset(vt[:, :, :, D:D + 1], 1.0)
        for c in range(NKC):
            raw = stage.tile([128, E], F32, tag="ld")
            nc.sync.dma_start(raw, t["value"][c * 128:(c + 1) * 128, :])
            nc.vector.tensor_copy(vt[:, c, :, 0:D],
                                  raw.rearrange("p (h d) -> p h d", h=H))

        # ---- main loop: per head pair, per k-chunk ----
        ex = {}       # h -> [128, NKC, SQ] bf16 exp tiles
        av_ps = {}    # h -> [65, SQ] f32 PSUM accumulator

        def sc_exp(h, c):
            bp, fc = 64 * (h % 2), h // 2
            ps = p_sc.tile([128, SQ], F32, tag="sc")
            for g in range(NQG):
                gsl = slice(g * QGS, (g + 1) * QGS)
                nc.tensor.matmul(ps[:, gsl],
                                 lhsT=kt[bp:bp + 64, fc, c * 128:(c + 1) * 128],
                                 rhs=xqa[bp:bp + 64, fc, gsl],
                                 start=True, stop=True)
            nc.scalar.activation(out=ex[h][:, c, :], in_=ps,
                                 func=mybir.ActivationFunctionType.Exp,
                                 bias=hbias[h][:, c:c + 1], scale=1.0)

        def av_mm(h, c):
            for g in range(NQG):
                gsl = slice(g * QGS, (g + 1) * QGS)
                nc.tensor.matmul(av_ps[h][:, gsl], lhsT=vt[:, c, h, :],
                                 rhs=ex[h][:, c, gsl],
                                 start=(c == 0), stop=(c == NKC - 1))

        def av_norm(h):
            # Copy AV+denominator out of PSUM (frees the accumulator early).
            # Denominator row -> [128, 8] via DMA so the reciprocal runs on
            # all DVE lanes, gather back to a q-ordered row, broadcast it
            # across 64 partitions with a rank-1 f32r matmul, multiply.
            # Odd heads hop to partition base 64 of avt via a small DMA.
            fc = h // 2
            avsb = small.tile([D + 1, SQ], F32, tag="avsb")
            nc.vector.tensor_copy(avsb, av_ps[h])
            del av_ps[h]
            # DRAM bounce: SBUF free-bytes can't be re-viewed as partitions,
            # but a flat DRAM row can. All APs here are plain strided reads/
            # writes (no partition-step-0 broadcasts) so RAW deps track.
            nc.gpsimd.dma_start(t["dscr"][h, :], avsb[D:D + 1, :])
            dn = small.tile([128, SQ // 128], F32, tag="dn")
            nc.gpsimd.dma_start(
                dn, t["dscr"][h, :].rearrange("(f p) -> p f", p=128))
            rc = small.tile([128, SQ // 128], F32, tag="rc")
            nc.vector.reciprocal(rc, dn)
            nc.gpsimd.dma_start(
                t["rscr"][h, :].rearrange("(f p) -> p f", p=128), rc)
            rrow = small.tile([1, SQ], F32, tag="rrow")
            nc.gpsimd.dma_start(rrow, t["rscr"][h:h + 1, :])
            pb = p_sc.tile([128, SQ], F32, tag="sc")
            for g in range(NQG):
                gsl = slice(g * QGS, (g + 1) * QGS)
                nc.tensor.matmul(pb[0:D, gsl], lhsT=ones1,
                                 rhs=rrow[0:1, gsl].bitcast(F32R),
                                 start=True, stop=True)
            if h % 2 == 0:
                nc.vector.tensor_tensor(out=avt[0:D, fc, :],
                                        in0=avsb[0:D, :], in1=pb[0:D, :],
                                        op=mult)
            else:
                avtmp = small.tile([64, SQ], BF16, tag="avtmp")
                nc.vector.tensor_tensor(out=avtmp, in0=avsb[0:D, :],
                                        in1=pb[0:D, :], op=mult)
                nc.gpsimd.dma_start(avt[64:64 + D, fc, :], avtmp)

        for p in range(H // 2):
            h0, h1 = 2 * p, 2 * p + 1
            ex[h0] = expp.tile([128, NKC, SQ], BF16, tag="exp", name=f"ex{h0}")
            ex[h1] = expp.tile([128, NKC, SQ], BF16, tag="exp", name=f"ex{h1}")
            for c in range(NKC):
                if c == 0 and p > 0:
                    # trailing AV chunk + normalization of the previous pair
                    av_mm(h0 - 2, NKC - 1)
                    av_mm(h1 - 2, NKC - 1)
                    av_norm(h0 - 2)
                    av_norm(h1 - 2)
                sc_exp(h0, c)
                sc_exp(h1, c)
                if c >= 1:
                    if c == 1:
                        av_ps[h0] = p_av.tile([D + 1, SQ], F32, tag="av",
                                              name=f"av{h0}")
                        av_ps[h1] = p_av.tile([D + 1, SQ], F32, tag="av",
                                              name=f"av{h1}")
                    av_mm(h0, c - 1)
                    av_mm(h1, c - 1)
        # tail: last chunk of AV for the final pair + norms
        av_mm(H - 2, NKC - 1)
        av_mm(H - 1, NKC - 1)
        av_norm(H - 2)
        av_norm(H - 1)

        # ---- output projection: 128-contraction accumulation over pairs ---
        for q_i in range(SQ // 128):
            qs = slice(q_i * 128, (q_i + 1) * 128)
            ps = p_sc.tile([128, E], F32, tag="sc")
            for j in range(4):
                nc.tensor.matmul(ps, lhsT=avt[:, j, qs], rhs=m_sb[:, j, :],
                                 start=(j == 0), stop=(j == 3))
            ob = outp.tile([128, E], F32, tag="ob")
            nc.vector.tensor_tensor(out=ob, in0=ps, in1=btot_rep, op=add)
            nc.sync.dma_start(t["out"][qs, :], ob)


def build_module(SQ, SK, has_qbias):
    nc = bacc.Bacc()
    t = {
        "query": nc.dram_tensor("query", [SQ, E], F32, kind="ExternalInput"),
        "key": nc.dram_tensor("key", [SK, E], F32, kind="ExternalInput"),
        "value": nc.dram_tensor("value", [SK, E], F32, kind="ExternalInput"),
        "mask": nc.dram_tensor("mask", [SK], I32, kind="ExternalInput"),
        "a2": nc.dram_tensor("a2", [128, 128], BF16, kind="ExternalInput"),
        "m2": nc.dram_tensor("m2", [128, 4, E], BF16, kind="ExternalInput"),
        "btot": nc.dram_tensor("btot", [E], F32, kind="ExternalInput"),
        "out": nc.dram_tensor("out", [SQ, E], F32, kind="ExternalOutput"),
        "dscr": nc.dram_tensor("dscr", [H, SQ], F32, kind="Internal"),
        "rscr": nc.dram_tensor("rscr", [H, SQ], F32, kind="Internal"),
    }
    if has_qbias:
        t["w2"] = nc.dram_tensor("w2", [128, 1], BF16, kind="ExternalInput")
    with tile.TileContext(nc) as tc:
        _emit(tc, t, SQ, SK, has_qbias)
    nc.compile()
    return nc


_MODULE_CACHE = {}


def _get_module(SQ, SK, has_qbias):
    key = (SQ, SK, has_qbias)
    if key not in _MODULE_CACHE:
        _MODULE_CACHE[key] = build_module(SQ, SK, has_qbias)
    return _MODULE_CACHE[key]


def _fold_weights(Wq, Wk, Wv, Wo, bv, bo):
    Wq, Wk, Wv, Wo = (np.asarray(w, np.float64) for w in (Wq, Wk, Wv, Wo))
    A = (Wq.T @ Wk) / np.sqrt(np.float64(D))
    a2 = np.zeros((128, 128), np.float64)     # blockdiag(A, A)
    a2[:D, :D] = A
    a2[D:, D:] = A
    a2 = a2.astype(ml_dtypes.bfloat16)
    Ms = [Wv.T @ Wo[:, h * D:(h + 1) * D].T for h in range(H)]
    # head-pair packing: head h at partitions 64*(h%2) .. +64, free slot h//2
    m2 = np.zeros((128, 4, E), np.float64)
    for h in range(H):
        m2[64 * (h % 2):64 * (h % 2) + D, h // 2, :] = Ms[h]
    m2 = m2.astype(ml_dtypes.bfloat16)
    btot = (np.asarray(bo, np.float64)
            + Wo @ np.tile(np.asarray(bv, np.float64), H)).astype(np.float32)
    return a2, m2, btot


def _run(inputs, trace=False):
    query = np.asarray(inputs["query"], np.float32)
    key = np.asarray(inputs["key"], np.float32)
    value = np.asarray(inputs["value"], np.float32)
    mask = np.asarray(inputs["mask"])
    a2, m2, btot = _fold_weights(inputs["Wq"], inputs["Wk"], inputs["Wv"],
                                 inputs["Wo"], inputs["bv"], inputs["bo"])
    bq = np.asarray(inputs["bq"], np.float64)
    has_qbias = bool(np.any(bq != 0))
    w2 = None
    if has_qbias:
        w2v = (np.asarray(inputs["Wk"], np.float64).T @ bq) / np.sqrt(float(D))
        w2 = np.concatenate([w2v, w2v]).reshape(128, 1).astype(ml_dtypes.bfloat16)

    n_batch, S = query.shape[0], query.shape[1]
    sq = S // 2

    # ---- key compaction: drop masked keys, pad to a common SKC ----
    idxs = [np.flatnonzero(mask[n, 0, 0, :] != 0) for n in range(n_batch)]
    maxk = max(int(ix.size) for ix in idxs)
    SKC = max(256, -(-maxk // 128) * 128)
    key_c = np.zeros((n_batch, SKC, E), np.float32)
    val_c = np.zeros((n_batch, SKC, E), np.float32)
    msk_c = np.zeros((n_batch, SKC), np.int32)
    for n, ix in enumerate(idxs):
        key_c[n, :ix.size] = key[n][ix]
        val_c[n, :ix.size] = value[n][ix]
        msk_c[n, :ix.size] = 1

    nc = _get_module(sq, SKC, has_qbias)

    in_maps = []
    for c in range(N_CORES):
        n, qh = divmod(c, 2)
        m = {
            "query": np.ascontiguousarray(query[n, qh * sq:(qh + 1) * sq, :]),
            "key": key_c[n],
            "value": val_c[n],
            "mask": msk_c[n],
            "a2": a2, "m2": m2, "btot": btot,
        }
        if has_qbias:
            m["w2"] = w2
        in_maps.append(m)

    res = run_bass_kernel_spmd(nc, in_maps, core_ids=list(range(N_CORES)),
                               trace=trace)
    out = np.empty((n_batch, S, E), np.float32)
    for c, r in enumerate(res.results):
        n, qh = divmod(c, 2)
        out[n, qh * sq:(qh + 1) * sq, :] = r["out"]
    return out, res


def kernel(**inputs) -> np.ndarray:
    out, _ = _run(inputs, trace=False)
    return out


# revision 19
# speedup vs baseline: 1.3046x; 1.3026x over previous
# Multi-head attention (N=4, S=2048, E=512, H=8, D=64) on 8 NeuronCores.
#
# Sharding: core c -> (batch n = c//2, query half qh = c%2). Each core
# computes attention for its 1024 query rows against the (compacted) keys of
# its batch, all 8 heads, and the full output projection for its rows, so
# outputs are disjoint and no collectives are needed.
#
# Key compaction: the mask is per-(batch, key) broadcast over queries/heads,
# and masked keys get exp(-1.25e8) == 0 exactly in f32, contributing nothing
# to the numerator or denominator. So keys with mask==0 are dropped on the
# host; all batches pad to a common chunk-multiple length SKC with pad keys
# biased to -inf. This halves score/exp/attn@V work for ~50% masks.
#
# Host-side weight folding (pure weight algebra, done once):
#   A  = Wq^T @ Wk / 8          scores = (Xq @ A) @ Xk^T  (raw K, one proj)
#   M_h = Wv^T @ Wo[:, h]^T     out += (attn @ Xv_h) @ M_h (Wv applied post)
#   btot = bo + Wo @ tile(bv,8) exact because attention rows sum to 1
#   bk-term cancels in softmax (constant over k); bq-term handled via a
#   per-k bias correction (zero for this problem's inputs).
#
# Device per core (head pair p owns heads 2p, 2p+1 at partition bases 0/64):
#   - PE-transpose Q,K chunks (bf16) to [e, s] layout
#   - xqa = blockdiag(A,A)^T-proj of Q^T per head pair (128-contraction)
#   - per (head, k-chunk): scores^T[k,q] on PE -> exp on ACT straight from
#     PSUM with per-partition mask bias -> bf16 ex tiles
#   - attn@V accumulates per chunk at 1-chunk lag (V carries a ones column
#     so softmax denominators ride along as PSUM row 64)
#   - normalization: denominator row DMA-rearranged to [128, 8] so the DVE
#     reciprocal runs on all lanes, broadcast back by DMA, one DVE multiply
#   - out[q,:] = sum_fc avt_fc^T @ M_fc (+btot): 128-contraction PSUM
#     accumulation over all four head pairs, one DVE add, DMA out

import numpy as np
import ml_dtypes

import concourse.bass as bass
import concourse.tile as tile
from concourse import bacc, mybir
from concourse.bass_utils import run_bass_kernel_spmd
from concourse.masks import make_identity

F32 = mybir.dt.float32
BF16 = mybir.dt.bfloat16
I32 = mybir.dt.int32

H = 8
D = 64
E = 512
N_CORES = 8
SQ = 1024             # per-core query rows
MASK_BIAS = -1.25e8   # == -1e9 / sqrt(64), applied pre-softmax


def _emit(tc, t, SQ, SK, has_qbias):
    nc = tc.nc
    NQC = SQ // 128           # query chunks (transpose granularity)
    NKC = SK // 128           # key chunks
    QGS = 512                 # q group size for matmul free dim (1 PSUM bank)
    NQG = SQ // QGS
    sub, mult, add = (mybir.AluOpType.subtract, mybir.AluOpType.mult,
                      mybir.AluOpType.add)

    with (
        tc.tile_pool(name="singles", bufs=1) as singles,
        tc.tile_pool(name="stage", bufs=3) as stage,
        tc.tile_pool(name="expp", bufs=4) as expp,
        tc.tile_pool(name="small", bufs=4) as small,
        tc.tile_pool(name="outp", bufs=2) as outp,
        tc.tile_pool(name="p_sc", bufs=2, space="PSUM") as p_sc,
        tc.tile_pool(name="p_av", bufs=2, space="PSUM") as p_av,
    ):
        # ---- constants / weights ----
        ident = singles.tile([128, 128], BF16)
        make_identity(nc, ident)

        a2 = singles.tile([128, 128], BF16)       # blockdiag(A, A)
        nc.sync.dma_start(a2, t["a2"][:])
        m_sb = singles.tile([128, 4, E], BF16)
        nc.sync.dma_start(m_sb, t["m2"][:])
        btot_rep = singles.tile([128, E], F32)
        nc.gpsimd.dma_start(btot_rep,
                            t["btot"][:][None, :].to_broadcast([128, E]))

        # mask -> additive bias, [128, NKC] with k = kt*128 + p
        mask_i = singles.tile([128, NKC], I32)
        nc.scalar.dma_start(mask_i, t["mask"][:].rearrange("(kt p) -> p kt", p=128))
        mask_f = singles.tile([128, NKC], F32)
        nc.vector.tensor_copy(mask_f, mask_i)
        mbias = singles.tile([128, NKC], F32)
        # (mask - 1) * (-MASK_BIAS):  mask=0 -> MASK_BIAS, mask=1 -> 0
        nc.vector.tensor_scalar(out=mbias, in0=mask_f, scalar1=1.0,
                                scalar2=-MASK_BIAS, op0=sub, op1=mult)

        # ---- persistent tensors ----
        F32R = mybir.dt.float32r
        ones1f = singles.tile([1, D], F32)
        nc.vector.memset(ones1f, 1.0)
        ones1 = singles.tile([1, D], F32R)       # f32r needs a rounding producer
        nc.vector.tensor_copy(ones1, ones1f)
        qt = singles.tile([128, 4, SQ], BF16)    # query^T: e=fc*128+p
        kt = singles.tile([128, 4, SK], BF16)    # key^T (compacted)
        vt = singles.tile([128, NKC, H, D + 1], BF16)  # value + ones col
        xqa = singles.tile([128, 4, SQ], BF16)   # (Xq @ A)^T per head
        # normalized (attn @ V)^T: head pair layout, odd heads at base 64
        avt = singles.tile([128, 4, SQ], BF16)

        # ---- chunk loader: load f32, cast bf16, PE-transpose into dst ----
        def load_transposed_chunk(src, dst, c, queue, cast_eng):
            raw = stage.tile([128, E], F32, tag="ld")
            queue.dma_start(raw, src[c * 128:(c + 1) * 128, :])
            cast = stage.tile([128, E], BF16, tag="cast")
            cast_eng.tensor_copy(cast, raw)
            tp = p_av.tile([128, 4, 128], BF16, tag="av")
            for eg in range(4):
                nc.tensor.transpose(tp[:, eg, :],
                                    cast[:, eg * 128:(eg + 1) * 128], ident)
            nc.vector.tensor_copy(dst[:, :, c * 128:(c + 1) * 128], tp)

        # ---- Q: load + transpose, then blockdiag-A projection per pair ----
        for c in range(NQC):
            load_transposed_chunk(t["query"][:], qt, c, nc.sync, nc.vector)
        for fc in range(4):
            for g in range(NQG):
                gsl = slice(g * QGS, (g + 1) * QGS)
                ps = p_av.tile([128, QGS], F32, tag="av")
                nc.tensor.matmul(ps, lhsT=a2, rhs=qt[:, fc, gsl],
                                 start=True, stop=True)
                nc.vector.tensor_copy(xqa[:, fc, gsl], ps)

        # ---- optional exact bq correction: per-(h,k) additive bias ----
        # scores^T gains (Xk_h @ (Wk^T bq / 8))[k], constant over q.
        if has_qbias:
            for c in range(NKC):
                load_transposed_chunk(t["key"][:], kt, c, nc.gpsimd, nc.gpsimd)
            w2 = singles.tile([128, 1], BF16)
            nc.sync.dma_start(w2, t["w2"][:])
            hbias = []
            for h in range(H):
                bp, fc = 64 * (h % 2), h // 2
                row = small.tile([1, SK], F32, tag="hb_row")
                for g in range(SK // 512):
                    ps = p_av.tile([128, 512], F32, tag="av")
                    nc.tensor.matmul(ps[0:1, :], lhsT=w2[bp:bp + 64, :],
                                     rhs=kt[bp:bp + 64, fc, g * 512:(g + 1) * 512],
                                     start=True, stop=True)
                    nc.vector.tensor_copy(row[:, g * 512:(g + 1) * 512],
                                          ps[0:1, :])
                hb = singles.tile([128, NKC], F32, name=f"hbias{h}")
                nc.gpsimd.dma_start(hb, row[0, :].rearrange("(kt p) -> p kt",
                                                            p=128))
                nc.vector.tensor_tensor(out=hb, in0=hb, in1=mbias, op=add)
                hbias.append(hb)
        else:
            hbias = [mbias] * H

        # ---- K: load + transpose (before the pair loop so the transpose
        # PSUM tiles never contend with the AV accumulators; pair-0 scores
        # start as soon as chunk 0 lands in kt) ----
        if not has_qbias:
            for c in range(NKC):
                load_transposed_chunk(t["key"][:], kt, c, nc.gpsimd,
                                      nc.vector)

        # ---- V: load + cast + ones column (consumed from pair 0's AV) ----
        nc.vector.memset(vt[:, :, :, D:D + 1], 1.0)
        for c in range(NKC):
            raw = stage.tile([128, E], F32, tag="ld")
            nc.sync.dma_start(raw, t["value"][c * 128:(c + 1) * 128, :])
            nc.vector.tensor_copy(vt[:, c, :, 0:D],
                                  raw.rearrange("p (h d) -> p h d", h=H))

        # ---- main loop: per head pair, per k-chunk ----
        ex = {}       # h -> [128, NKC, SQ] bf16 exp tiles
        av_ps = {}    # h -> [65, SQ] f32 PSUM accumulator

        def sc_exp(h, c):
            bp, fc = 64 * (h % 2), h // 2
            ps = p_sc.tile([128, SQ], F32, tag="sc")
            for g in range(NQG):
                gsl = slice(g * QGS, (g + 1) * QGS)
                nc.tensor.matmul(ps[:, gsl],
                                 lhsT=kt[bp:bp + 64, fc, c * 128:(c + 1) * 128],
                                 rhs=xqa[bp:bp + 64, fc, gsl],
                                 start=True, stop=True)
            nc.scalar.activation(out=ex[h][:, c, :], in_=ps,
                                 func=mybir.ActivationFunctionType.Exp,
                                 bias=hbias[h][:, c:c + 1], scale=1.0)

        def av_mm(h, c):
            for g in range(NQG):
                gsl = slice(g * QGS, (g + 1) * QGS)
                nc.tensor.matmul(av_ps[h][:, gsl], lhsT=vt[:, c, h, :],
                                 rhs=ex[h][:, c, gsl],
                                 start=(c == 0), stop=(c == NKC - 1))

        def av_norm(h):
            # Copy AV+denominator out of PSUM (frees the accumulator early).
            # Denominator row -> [128, 8] via DMA so the reciprocal runs on
            # all DVE lanes, gather back to a q-ordered row, broadcast it
            # across 64 partitions with a rank-1 f32r matmul, multiply.
            # Odd heads hop to partition base 64 of avt via a small DMA.
            fc = h // 2
            avsb = small.tile([D + 1, SQ], F32, tag="avsb")
            nc.vector.tensor_copy(avsb, av_ps[h])
            del av_ps[h]
            # DRAM bounce: SBUF free-bytes can't be re-viewed as partitions,
            # but a flat DRAM row can. All APs here are plain strided reads/
            # writes (no partition-step-0 broadcasts) so RAW deps track.
            nc.gpsimd.dma_start(t["dscr"][h, :], avsb[D:D + 1, :])
            dn = small.tile([128, SQ // 128], F32, tag="dn")
            nc.gpsimd.dma_start(
                dn, t["dscr"][h, :].rearrange("(f p) -> p f", p=128))
            rc = small.tile([128, SQ // 128], F32, tag="rc")
            nc.vector.reciprocal(rc, dn)
            nc.gpsimd.dma_start(
                t["rscr"][h, :].rearrange("(f p) -> p f", p=128), rc)
            rrow = small.tile([1, SQ], F32, tag="rrow")
            nc.gpsimd.dma_start(rrow, t["rscr"][h:h + 1, :])
            pb = p_av.tile([D, SQ], F32, tag="av")
            for g in range(NQG):
                gsl = slice(g * QGS, (g + 1) * QGS)
                nc.tensor.matmul(pb[0:D, gsl], lhsT=ones1,
                                 rhs=rrow[0:1, gsl].bitcast(F32R),
                                 start=True, stop=True)
            if h % 2 == 0:
                nc.vector.tensor_tensor(out=avt[0:D, fc, :],
                                        in0=avsb[0:D, :], in1=pb[0:D, :],
                                        op=mult)
            else:
                avtmp = small.tile([64, SQ], BF16, tag="avtmp")
                nc.vector.tensor_tensor(out=avtmp, in0=avsb[0:D, :],
                                        in1=pb[0:D, :], op=mult)
                nc.gpsimd.dma_start(avt[64:64 + D, fc, :], avtmp)

        for p in range(H // 2):
            h0, h1 = 2 * p, 2 * p + 1
            ex[h0] = expp.tile([128, NKC, SQ], BF16, tag="exp", name=f"ex{h0}")
            ex[h1] = expp.tile([128, NKC, SQ], BF16, tag="exp", name=f"ex{h1}")
            for c in range(NKC):
                sc_exp(h0, c)
                sc_exp(h1, c)
                if c >= 1:
                    if c == 1:
                        av_ps[h0] = p_av.tile([D + 1, SQ], F32, tag="av",
                                              name=f"av{h0}")
                        av_ps[h1] = p_av.tile([D + 1, SQ], F32, tag="av",
                                              name=f"av{h1}")
                    av_mm(h0, c - 1)
                    av_mm(h1, c - 1)
            # pair tail: last AV chunk + normalization, emitted before the
            # next pair so the norm DMA chains overlap its score pipeline
            av_mm(h0, NKC - 1)
            av_norm(h0)
            av_mm(h1, NKC - 1)
            av_norm(h1)

        # ---- output projection: 128-contraction accumulation over pairs ---
        for q_i in range(SQ // 128):
            qs = slice(q_i * 128, (q_i + 1) * 128)
            ps = p_sc.tile([128, E], F32, tag="sc")
            for j in range(4):
                nc.tensor.matmul(ps, lhsT=avt[:, j, qs], rhs=m_sb[:, j, :],
                                 start=(j == 0), stop=(j == 3))
            ob = outp.tile([128, E], F32, tag="ob")
            nc.vector.tensor_tensor(out=ob, in0=ps, in1=btot_rep, op=add)
            nc.sync.dma_start(t["out"][qs, :], ob)


def build_module(SQ, SK, has_qbias):
    nc = bacc.Bacc()
    t = {
        "query": nc.dram_tensor("query", [SQ, E], F32, kind="ExternalInput"),
        "key": nc.dram_tensor("key", [SK, E], F32, kind="ExternalInput"),
        "value": nc.dram_tensor("value", [SK, E], F32, kind="ExternalInput"),
        "mask": nc.dram_tensor("mask", [SK], I32, kind="ExternalInput"),
        "a2": nc.dram_tensor("a2", [128, 128], BF16, kind="ExternalInput"),
        "m2": nc.dram_tensor("m2", [128, 4, E], BF16, kind="ExternalInput"),
        "btot": nc.dram_tensor("btot", [E], F32, kind="ExternalInput"),
        "out": nc.dram_tensor("out", [SQ, E], F32, kind="ExternalOutput"),
        "dscr": nc.dram_tensor("dscr", [H, SQ], F32, kind="Internal"),
        "rscr": nc.dram_tensor("rscr", [H, SQ], F32, kind="Internal"),
    }
    if has_qbias:
        t["w2"] = nc.dram_tensor("w2", [128, 1], BF16, kind="ExternalInput")
    with tile.TileContext(nc) as tc:
        _emit(tc, t, SQ, SK, has_qbias)
    nc.compile()
    return nc


_MODULE_CACHE = {}


def _get_module(SQ, SK, has_qbias):
    key = (SQ, SK, has_qbias)
    if key not in _MODULE_CACHE:
        _MODULE_CACHE[key] = build_module(SQ, SK, has_qbias)
    return _MODULE_CACHE[key]


def _fold_weights(Wq, Wk, Wv, Wo, bv, bo):
    Wq, Wk, Wv, Wo = (np.asarray(w, np.float64) for w in (Wq, Wk, Wv, Wo))
    A = (Wq.T @ Wk) / np.sqrt(np.float64(D))
    a2 = np.zeros((128, 128), np.float64)     # blockdiag(A, A)
    a2[:D, :D] = A
    a2[D:, D:] = A
    a2 = a2.astype(ml_dtypes.bfloat16)
    Ms = [Wv.T @ Wo[:, h * D:(h + 1) * D].T for h in range(H)]
    # head-pair packing: head h at partitions 64*(h%2) .. +64, free slot h//2
    m2 = np.zeros((128, 4, E), np.float64)
    for h in range(H):
        m2[64 * (h % 2):64 * (h % 2) + D, h // 2, :] = Ms[h]
    m2 = m2.astype(ml_dtypes.bfloat16)
    btot = (np.asarray(bo, np.float64)
            + Wo @ np.tile(np.asarray(bv, np.float64), H)).astype(np.float32)
    return a2, m2, btot


def _run(inputs, trace=False):
    query = np.asarray(inputs["query"], np.float32)
    key = np.asarray(inputs["key"], np.float32)
    value = np.asarray(inputs["value"], np.float32)
    mask = np.asarray(inputs["mask"])
    a2, m2, btot = _fold_weights(inputs["Wq"], inputs["Wk"], inputs["Wv"],
                                 inputs["Wo"], inputs["bv"], inputs["bo"])
    bq = np.asarray(inputs["bq"], np.float64)
    has_qbias = bool(np.any(bq != 0))
    w2 = None
    if has_qbias:
        w2v = (np.asarray(inputs["Wk"], np.float64).T @ bq) / np.sqrt(float(D))
        w2 = np.concatenate([w2v, w2v]).reshape(128, 1).astype(ml_dtypes.bfloat16)

    n_batch, S = query.shape[0], query.shape[1]
    sq = S // 2

    # ---- key compaction: drop masked keys, pad to a common SKC ----
    idxs = [np.flatnonzero(mask[n, 0, 0, :] != 0) for n in range(n_batch)]
    maxk = max(int(ix.size) for ix in idxs)
    SKC = max(256, -(-maxk // 128) * 128)
    key_c = np.zeros((n_batch, SKC, E), np.float32)
    val_c = np.zeros((n_batch, SKC, E), np.float32)
    msk_c = np.zeros((n_batch, SKC), np.int32)
    for n, ix in enumerate(idxs):
        key_c[n, :ix.size] = key[n][ix]
        val_c[n, :ix.size] = value[n][ix]
        msk_c[n, :ix.size] = 1

    nc = _get_module(sq, SKC, has_qbias)

    in_maps = []
    for c in range(N_CORES):
        n, qh = divmod(c, 2)
        m = {
            "query": np.ascontiguousarray(query[n, qh * sq:(qh + 1) * sq, :]),
            "key": key_c[n],
            "value": val_c[n],
            "mask": msk_c[n],
            "a2": a2, "m2": m2, "btot": btot,
        }
        if has_qbias:
            m["w2"] = w2
        in_maps.append(m)

    res = run_bass_kernel_spmd(nc, in_maps, core_ids=list(range(N_CORES)),
                               trace=trace)
    out = np.empty((n_batch, S, E), np.float32)
    for c, r in enumerate(res.results):
        n, qh = divmod(c, 2)
        out[n, qh * sq:(qh + 1) * sq, :] = r["out"]
    return out, res


def kernel(**inputs) -> np.ndarray:
    out, _ = _run(inputs, trace=False)
    return out
